# revision 1
# baseline (speedup 1.0000x reference)
"""Trainium2 Bass kernel for nn_DenseAttentionOneHead (B=2, L=4096, H=1024).

Reference math:
    h   = hidden * cos + rotate_half(hidden) * sin      (RoPE)
    q   = h @ W_q.T
    out = (q @ h^T) @ h                                 (no softmax)

With no softmax the L x L score matrix factorizes away, and W_q folds in:
    out[b] = h[b] @ M[b],  M[b] = W_q^T G[b],  G[b] = h[b].T @ h[b]  (H x H)

Sharding (8 NeuronCores): cores 0-3 own batch 0's four 1024-row L-chunks,
cores 4-7 batch 1. Each core computes a partial G over its chunk. Instead
of a 4MB fp32 AllReduce of G (cost-model ~225us), the cross-core exchange
is a bf16 ReduceScatter of G (0.5MB out, ~28us) -> each core computes its
256-column slice of M = W_q^T G -> bf16 AllGather of M (2MB out, ~67us).
G is symmetric, so a row-shard of G is a column-shard: the bounce buffers
are laid out shard-major ([rank][k][col-within-shard]) so both the
RS output and the AG output arrive in exactly the layout the next matmul
wants -- no transposes or reshuffles on the critical path.

Engine plan per core: DVE does RoPE tiles 0-6 (+ W/M dequants), Pool does
RoPE tile 7; PE pipelines G's first column-half into 8 persistent PSUM
accumulators while RoPE streams, drains them interleaved with the second
column-half pass, does the h^T transposes (4 per PSUM bank, one wide copy)
under the RS, then the M-slice and y matmuls. No-reader self-matmuls keep
the PE p-state ramped through both collective windows (the cost model
prices post-idle matmuls up to 3.7x slower). ACT/DVE split the PSUM->SBUF
copies; the two HWDGE rings split the traffic (h/G-out/M-in-write/y-even
on sync, cos/sin/W/G-slice/M-fetch/y-odd on scalar).
"""

import os

import numpy as np

import jax

try:
    _cache_dir = os.path.join(os.path.expanduser("~"), ".cache", "bass_kernel_jax")
    os.makedirs(_cache_dir, exist_ok=True)
    jax.config.update("jax_compilation_cache_dir", _cache_dir)
    jax.config.update("jax_persistent_cache_min_compile_time_secs", 1.0)
except Exception:
    pass

import concourse.bacc as bacc
import concourse.mybir as mybir
import concourse.tile as tile
from concourse import masks
from concourse.bass_utils import run_bass_kernel_spmd

F32 = mybir.dt.float32
F32R = mybir.dt.float32r
BF16 = mybir.dt.bfloat16

B, L, H = 2, 4096, 1024
L_CHUNK = 1024
HH = H // 2
NT = L_CHUNK // 128   # 8 row tiles per chunk
MT = H // 128         # 8 column tiles
NSH = H // 4          # 256: columns per RS shard
GROUPS = [[0, 1, 2, 3], [4, 5, 6, 7]]
NACC = 8              # PSUM bank ring ("ps" tag)
WARM1 = 124           # PE keep-warm matmuls during the ReduceScatter
WARM2 = 359           # PE keep-warm matmuls during the AllGather


def _emit_once(nc, tc, h_d, c1_d, s1_d, wq_d, y_d):
    h_ap = h_d.ap().rearrange("(t p) c -> p t c", p=128)
    c1_ap = c1_d.ap().rearrange("(t p) c -> p t c", p=128)
    s1_ap = s1_d.ap().rearrange("(t p) c -> p t c", p=128)
    wq_ap = wq_d.ap().rearrange("(t p) c -> p t c", p=128)
    y_ap = y_d.ap().rearrange("(t p) c -> p t c", p=128)

    with (
        tc.tile_pool(name="persist", bufs=1) as persist,
        tc.tile_pool(name="stream", bufs=1) as stream,
        tc.tile_pool(name="pacc", bufs=NACC, space="PSUM") as pacc,
        tc.tile_pool(name="dram", bufs=1, space="DRAM") as dram,
    ):
        hr = persist.tile([128, NT, H], F32R, name="hr")
        hrt = persist.tile([128, MT, L_CHUNK], F32R, name="hrt")
        wq_b = persist.tile([128, MT, H], BF16, name="wq_b")
        mr = persist.tile([128, MT, H], F32R, name="mr")
        gsl = persist.tile([128, MT, NSH], BF16, name="gsl")

        # DRAM bounce buffers, shard-major layout:
        #   g_in  flat = rank*262144 + k*256 + nw   <->  G_part[k, 256*rank+nw]
        #   g_rs  flat = k*256 + nw                 <->  G[k, 256*myrank+nw]
        #   m_in  flat = hcol*256 + nw              <->  M[hcol, 256*myrank+nw]
        #   m_out flat = rank*262144 + h*256 + nw   <->  M[h, 256*rank+nw]
        g_in = dram.tile([128, MT * H], BF16, name="g_in")
        g_rs = dram.tile([32, MT * H], BF16, name="g_rs")
        m_in = dram.tile([32, MT * H], BF16, name="m_in")
        m_out = dram.tile([128, MT * H], BF16, name="m_out")
        gv = g_in[:].rearrange("(r kp) (kq nw) -> kp kq r nw", r=4, kq=32)
        # one-DMA fetch views: [pp, kt, ...] with k = kt*128 + pp
        rv = (
            g_rs[:]
            .rearrange("p (kq nw) -> (p kq) nw", kq=32)
            .rearrange("(kt pp) nw -> pp kt nw", pp=128)
        )
        mv = (
            m_in[:]
            .rearrange("p (kq nw) -> (p kq) nw", kq=32)
            .rearrange("(mt pp) nw -> pp mt nw", pp=128)
        )
        ov = m_out[:].rearrange("(r kp) (kq nw) -> kp kq r nw", r=4, kq=32)

        # persistent PSUM accumulators for G[:, 0:512], all 8 row bands
        gacc = [
            pacc.tile([128, 512], F32, name=f"gacc{i}", tag="ps")
            for i in range(MT)
        ]

        # ---- RoPE + pipelined G first-half. Tiles 0-6 stream on DVE;
        # tile 7 runs on the otherwise-idle Pool engine so the two engines
        # finish together. ----
        for t in range(NT):
            ht = stream.tile([128, H], F32, name="ht", tag="ld1024", bufs=5)
            ct = stream.tile([128, HH], F32, name="ct", tag="ldc", bufs=5)
            st = stream.tile([128, HH], F32, name="st", tag="lds", bufs=5)
            nc.sync.dma_start(ht[:], h_ap[:, t, :])
            nc.scalar.dma_start(ct[:], c1_ap[:, t, :])
            nc.scalar.dma_start(st[:], s1_ap[:, t, :])
            h1 = ht[:, 0:HH]
            h2 = ht[:, HH:H]
            tag = "tmp" if t < 7 else "tmpp"
            m1 = stream.tile([128, HH], F32, name="m1", tag=tag, bufs=4)
            m2 = stream.tile([128, HH], F32, name="m2", tag=tag, bufs=4)
            m3 = stream.tile([128, HH], F32, name="m3", tag=tag, bufs=4)
            m4 = stream.tile([128, HH], F32, name="m4", tag=tag, bufs=4)
            eng = nc.vector if t < 7 else nc.gpsimd
            eng.tensor_mul(m1[:], h1, ct[:])
            eng.tensor_mul(m2[:], h2, st[:])
            eng.tensor_sub(hr[:, t, 0:HH], m1[:], m2[:])
            eng.tensor_mul(m3[:], h2, ct[:])
            eng.tensor_mul(m4[:], h1, st[:])
            eng.tensor_add(hr[:, t, HH:H], m3[:], m4[:])
            # G[:, 0:512] accumulation rides the RoPE stream
            for mt in range(MT):
                nc.tensor.matmul(
                    gacc[mt][:],
                    hr[:, t, mt * 128:(mt + 1) * 128],
                    hr[:, t, 0:512],
                    start=(t == 0),
                    stop=(t == NT - 1),
                )

        ident_f = stream.tile([128, 128], F32, name="ident_f", tag="identf")
        masks.make_identity(nc, ident_f[:])
        ident = stream.tile([128, 128], F32R, name="ident", tag="ident")
        nc.vector.tensor_copy(ident[:], ident_f[:])

        # W_q load (scalar ring, behind c/s) + bf16 pack on DVE (idle
        # post-RoPE; keeps the ACT queue clear for the G pack copies)
        for mt in range(MT):
            wt = stream.tile([128, H], F32, name="wt", tag="ldw", bufs=2)
            nc.scalar.dma_start(wt[:], wq_ap[:, mt, :])
            nc.vector.tensor_copy(wq_b[:, mt, :], wt[:])

        # drain the pipelined accumulators while computing G cols 512:1024;
        # both halves of a row band pack into one [128,1024] tile -> one DMA
        # (the shard-major dst AP merges to 3 dims for full-width rows).
        for mt in range(MT):
            gb2 = stream.tile([128, H], BF16, name="gb2", tag="gb", bufs=3)
            nc.scalar.copy(gb2[:, 0:512], gacc[mt][:])
            ps = pacc.tile([128, 512], F32, name="ps1", tag="ps")
            for kt in range(NT):
                nc.tensor.matmul(
                    ps[:],
                    hr[:, kt, mt * 128:(mt + 1) * 128],
                    hr[:, kt, 512:1024],
                    start=(kt == 0),
                    stop=(kt == NT - 1),
                )
            nc.scalar.copy(gb2[:, 512:1024], ps[:])
            nc.sync.dma_start(gv[4 * mt:4 * (mt + 1), :, :, :], gb2[:])

        # ReduceScatter of partial G (bf16): each core gets its 256 columns
        nc.gpsimd.collective_compute(
            "ReduceScatter",
            mybir.AluOpType.add,
            replica_groups=GROUPS,
            ins=[g_in[:]],
            outs=[g_rs[:]],
        )

        # transposes for the y-phase stationary h^T (PE idles during RS).
        # 4 transposes share one PSUM bank + one wide ACT copy, so the
        # PE<->ACT semaphore round-trip amortizes 4x.
        for mt in range(MT):
            for ta in range(0, NT, 4):
                pstb = pacc.tile([128, 512], F32R, name="pstb", tag="ps")
                for j in range(4):
                    nc.tensor.transpose(
                        pstb[:, j * 128:(j + 1) * 128],
                        hr[:, ta + j, mt * 128:(mt + 1) * 128],
                        ident[:],
                    )
                nc.vector.tensor_copy(
                    hrt[:, mt, ta * 128:(ta + 4) * 128], pstb[:]
                )

        # self-matmuls with no readers keep the PE p-state ramped through the
        # RS wait (cost model prices post-idle matmuls up to 3.7x slower)
        js = pacc.tile([128, 512], F32, name="js", tag="ps")
        for _ in range(WARM1):
            nc.tensor.matmul(
                js[:], hr[:, 0, 0:128], hr[:, 0, 0:512],
                start=True, stop=True, skip_group_check=True,
            )

        # fetch my G column slice (already [k, nw] matmul layout), one DMA
        nc.scalar.dma_start(gsl[:, :, :], rv[:, :, :])

        # M slice = W_q^T G[:, mycols]  (bf16 matmuls, fp32 PSUM)
        mqall = persist.tile([128, MT, NSH], BF16, name="mqall")
        for mt in range(MT):
            ps = pacc.tile([128, NSH], F32, name="psm", tag="ps")
            for dk in range(MT):
                nc.tensor.matmul(
                    ps[:],
                    wq_b[:, dk, mt * 128:(mt + 1) * 128],
                    gsl[:, dk, :],
                    start=(dk == 0),
                    stop=(dk == MT - 1),
                )
            nc.scalar.copy(mqall[:, mt, :], ps[:])
        nc.sync.dma_start(mv[:, :, :], mqall[:, :, :])

        # AllGather of M slices (bf16)
        nc.gpsimd.collective_compute(
            "AllGather",
            mybir.AluOpType.bypass,
            replica_groups=GROUPS,
            ins=[m_in[:]],
            outs=[m_out[:]],
        )

        # keep the PE warm through the AllGather idle window
        js2 = pacc.tile([128, 512], F32, name="js2", tag="ps")
        for _ in range(WARM2):
            nc.tensor.matmul(
                js2[:], hr[:, 0, 0:128], hr[:, 0, 0:512],
                start=True, stop=True, skip_group_check=True,
            )

        # fetch + dequant M per-tile (DVE), pipelined with the y matmuls
        for ht in range(MT):
            mf = stream.tile([128, H], BF16, name="mf", tag="mf", bufs=3)
            nc.scalar.dma_start(mf[:], ov[4 * ht:4 * (ht + 1), :, :, :])
            nc.vector.tensor_copy(mr[:, ht, :], mf[:])

        # y = h @ M. First half k-outer (starts on the first dequanted M
        # tile); second half lt-major so the 8 accumulator stops stagger and
        # the output copies/DMAs drain under the remaining matmuls instead
        # of all queueing after the last one.
        def y_out(lt, nh2, ps):
            yo = stream.tile([128, 512], F32, name="yo", tag="yo", bufs=4)
            if lt % 2 == 0:
                nc.scalar.copy(yo[:], ps[:])
                nc.sync.dma_start(
                    y_ap[:, lt, nh2 * 512:(nh2 + 1) * 512], yo[:]
                )
            else:
                nc.vector.tensor_copy(yo[:], ps[:])
                nc.scalar.dma_start(
                    y_ap[:, lt, nh2 * 512:(nh2 + 1) * 512], yo[:]
                )

        psy = [
            pacc.tile([128, 512], F32, name=f"psy{lt}", tag="ps")
            for lt in range(NT)
        ]
        for ht in range(MT):
            for lt in range(NT):
                nc.tensor.matmul(
                    psy[lt][:],
                    hrt[:, ht, lt * 128:(lt + 1) * 128],
                    mr[:, ht, 0:512],
                    start=(ht == 0),
                    stop=(ht == MT - 1),
                )
        for lt in range(NT):
            y_out(lt, 0, psy[lt])
        for lt in range(NT):
            ps2 = pacc.tile([128, 512], F32, name="psy2", tag="ps")
            for ht in range(MT):
                nc.tensor.matmul(
                    ps2[:],
                    hrt[:, ht, lt * 128:(lt + 1) * 128],
                    mr[:, ht, 512:1024],
                    start=(ht == 0),
                    stop=(ht == MT - 1),
                )
            y_out(lt, 1, ps2)


_NC_CACHE = {}


def _build():
    if "nc" in _NC_CACHE:
        return _NC_CACHE["nc"]
    nc = bacc.Bacc("TRN2", target_bir_lowering=False, debug=False, num_devices=8)
    h_d = nc.dram_tensor("h", [L_CHUNK, H], F32, kind="ExternalInput")
    c1_d = nc.dram_tensor("c1", [L_CHUNK, HH], F32, kind="ExternalInput")
    s1_d = nc.dram_tensor("s1", [L_CHUNK, HH], F32, kind="ExternalInput")
    wq_d = nc.dram_tensor("wq", [H, H], F32, kind="ExternalInput")
    y_d = nc.dram_tensor("y", [L_CHUNK, H], F32, kind="ExternalOutput")
    with tile.TileContext(nc) as tc:
        _emit_once(nc, tc, h_d, c1_d, s1_d, wq_d, y_d)
    nc.compile()
    _NC_CACHE["nc"] = nc
    return nc


def kernel(hidden_states, W_q, cos, sin):
    hs = np.asarray(hidden_states, dtype=np.float32)
    wq = np.ascontiguousarray(np.asarray(W_q, dtype=np.float32))
    cos = np.asarray(cos, dtype=np.float32)
    sin = np.asarray(sin, dtype=np.float32)
    in_maps = []
    for core in range(8):
        b, i = core // 4, core % 4
        sl = slice(i * L_CHUNK, (i + 1) * L_CHUNK)
        in_maps.append({
            "h": np.ascontiguousarray(hs[b, sl]),
            "c1": np.ascontiguousarray(cos[sl, :HH]),
            "s1": np.ascontiguousarray(sin[sl, :HH]),
            "wq": wq,
        })

    nc = _build()
    res = run_bass_kernel_spmd(nc, in_maps, core_ids=list(range(8)))

    out = np.empty((B, L, H), dtype=np.float32)
    for core, r in enumerate(res.results):
        b, i = core // 4, core % 4
        out[b, i * L_CHUNK:(i + 1) * L_CHUNK] = r["y"]
    return out



# revision 7
# speedup vs baseline: 1.4470x; 1.4470x over previous
"""Trainium2 Bass kernel for nn_DenseAttentionOneHead (B=2, L=4096, H=1024).

Reference math:
    h   = hidden * cos + rotate_half(hidden) * sin      (RoPE)
    q   = h @ W_q.T
    out = (q @ h^T) @ h                                 (no softmax)

With no softmax the L x L score matrix factorizes away and W_q folds in:
    out[b] = h[b] @ M[b],  M[b] = W_q^T G[b],  G[b] = h[b]^T h[b]  (H x H)

Sharding (8 NeuronCores, ZERO collectives): cores are column-parallel
inside each batch. Core (b, j) computes a 256-column slice of G, M and y
for batch b with the FULL L=4096 contraction, reading all of h[b]. The
cost-model collectives (15us fixed + bytes/40GBps each) priced the old
row-parallel ReduceScatter+AllGather exchange at ~95us of serial time;
re-reading h (8MB bf16, ~23us of DMA overlapped under compute) is far
cheaper.

All 8 cores run one SPMD module: the host rolls the RoPE *pair* axis by
128*j so every core's slice lands at permuted columns {0:128, 512:640},
and permutes W_q on both axes to match (contractions are order-agnostic).
The host un-permutes the output columns when scattering back.

Per-core phases (PE totals 180K cycles = 75us at 2.4GHz):
  1. Stream 32 l-tiles of packed [h|cos|sin] (one 4KB-desc DMA each);
     RoPE on DVE+Pool via scalar_tensor_tensor (4x DVE mode, 194ns/op);
     accumulate G[:, slice] into 4 persistent PSUM banks; PE-transpose
     each RoPE'd tile into hT (bf16, 1 cyc/row) for phase 3; ACT drains
     the transpose banks. One 512-row PE filler per tile bridges the
     ~180ns/tile DMA-vs-PE deficit so the PE p-state never drops.
  2. M[:, slice] = W_q^T G[:, slice]: 64 matmuls, W arriving right
     behind the h stream on the same DMA ring.
  3. y[:, slice] = h @ M[:, slice]: 256 matmuls from hT against M,
     PSUM drained via DVE/ACT casts to bf16 and DMA'd out.
"""

import os

import numpy as np

import jax

try:
    _cache_dir = os.path.join(os.path.expanduser("~"), ".cache", "bass_kernel_jax")
    os.makedirs(_cache_dir, exist_ok=True)
    jax.config.update("jax_compilation_cache_dir", _cache_dir)
    jax.config.update("jax_persistent_cache_min_compile_time_secs", 1.0)
except Exception:
    pass

import ml_dtypes

import concourse.bacc as bacc
import concourse.mybir as mybir
import concourse.tile as tile
from concourse import masks
from concourse.bass_utils import run_bass_kernel_spmd

F32 = mybir.dt.float32
BF16 = mybir.dt.bfloat16
BF16_NP = ml_dtypes.bfloat16

B, L, H = 2, 4096, 1024
HH = H // 2          # 512 RoPE pairs
NT = L // 128        # 32 l-tiles
NCS = 256            # columns of G/M/y per core
PAIRS = 128          # RoPE pairs per core slice

BYP = mybir.AluOpType.bypass
MUL = mybir.AluOpType.mult
ADD = mybir.AluOpType.add
SUB = mybir.AluOpType.subtract


def _emit_once(nc, tc, hcs_d, wq_d, y_d):
    # hcs rows are l; cols = [h_perm (1024) | cos_perm (512) | sin_perm (512)]
    hcs_ap = hcs_d.ap().rearrange("(t p) c -> p t c", p=128)  # [128, 32, 2048]
    wq_ap = wq_d.ap().rearrange("(t p) c -> p t c", p=128)    # [128, 8, 1024]
    y_ap = y_d.ap().rearrange("(t p) c -> p t c", p=128)      # [128, 32, 256]

    with (
        tc.tile_pool(name="persist", bufs=1) as persist,
        tc.tile_pool(name="stream", bufs=1) as stream,
        tc.tile_pool(name="pacc", bufs=1, space="PSUM") as pacc,
    ):
        hT = persist.tile([128, 8, L], BF16, name="hT")        # h^T, 64KB/part
        wq_sb = persist.tile([128, 8, H], BF16, name="wq_sb")  # W_q, 16KB/part
        gsl = persist.tile([128, 8, NCS], BF16, name="gsl")    # G[:, slice]
        mq = persist.tile([128, 8, NCS], BF16, name="mq")      # M[:, slice]
        dum = persist.tile([128, 512], BF16, name="dum")       # filler operand
        nc.vector.memset(dum[:], 0.0)

        identf = stream.tile([128, 128], F32, name="identf")
        masks.make_identity(nc, identf[:])
        identb = stream.tile([128, 128], BF16, name="identb")
        nc.vector.tensor_copy(identb[:], identf[:])

        fill = pacc.tile([128, 512], F32, name="fill", tag="fill", bufs=1)

        def fillers(n):
            # no-reader self-matmuls keep the PE p-state ramped across
            # short DMA/copy waits (post-idle matmuls cost up to 2x)
            for _ in range(n):
                nc.tensor.matmul(
                    fill[:], dum[:, 0:128], dum[:],
                    start=True, stop=True, skip_group_check=True,
                )

        # G[:, slice] accumulators: 4 banks, 2 row-blocks each, packed
        # [ob_even: lo|hi, ob_odd: lo|hi] = [0:128|128:256, 256:384|384:512]
        gacc = [
            pacc.tile([128, 512], F32, name=f"gacc{i}", tag="acc", bufs=4)
            for i in range(4)
        ]

        # ---- phase 1: stream h|c|s, RoPE, G accumulation, transposes ----
        for t in range(NT):
            hc = stream.tile([128, 2048], BF16, name="hc", tag="ld", bufs=4)
            nc.sync.dma_start(hc[:], hcs_ap[:, t, :])
            h1 = hc[:, 0:HH]
            h2 = hc[:, HH:H]
            ct = hc[:, H:H + HH]
            st = hc[:, H + HH:H + 2 * HH]
            hr = stream.tile([128, H], BF16, name="hr", tag="hr", bufs=6)
            m1 = stream.tile([128, HH], BF16, name="m1", tag="m1", bufs=2)
            m2 = stream.tile([128, HH], BF16, name="m2", tag="m2", bufs=2)
            m3 = stream.tile([128, HH], BF16, name="m3", tag="m3", bufs=2)
            m4 = stream.tile([128, HH], BF16, name="m4", tag="m4", bufs=2)
            # RoPE: scalar_tensor_tensor with bypassed scalar runs the
            # plain tensor-tensor ALU op in the DVE 4x perf mode (194ns)
            nc.vector.scalar_tensor_tensor(m1[:], h1, 0.0, ct, BYP, MUL)
            nc.vector.scalar_tensor_tensor(m2[:], h2, 0.0, st, BYP, MUL)
            nc.vector.scalar_tensor_tensor(m3[:], h2, 0.0, ct, BYP, MUL)
            nc.vector.scalar_tensor_tensor(m4[:], h1, 0.0, st, BYP, MUL)
            nc.vector.scalar_tensor_tensor(hr[:, 0:HH], m1[:], 0.0, m2[:], BYP, SUB)
            nc.gpsimd.tensor_add(hr[:, HH:H], m3[:], m4[:])

            if t >= 2:
                fillers(1)
            # G[:, slice] += hr^T hr[:, slice]; slice = {0:128, 512:640}.
            # HW: start=True zeroes the WHOLE psum bank, so only the first
            # group touching a bank starts; co-resident groups accumulate
            # onto the zeroed bank with start=False.
            for ob in range(8):
                acc = gacc[ob // 2]
                base = (ob % 2) * 256
                lhs = hr[:, ob * 128:(ob + 1) * 128]
                nc.tensor.matmul(
                    acc[:, base:base + 128], lhs, hr[:, 0:PAIRS],
                    start=(t == 0 and ob % 2 == 0), stop=(t == NT - 1),
                    skip_group_check=True,
                )
                nc.tensor.matmul(
                    acc[:, base + 128:base + 256], lhs, hr[:, HH:HH + PAIRS],
                    start=False, stop=(t == NT - 1),
                    skip_group_check=True,
                )
            # hT[:, :, t*128:(t+1)*128] = hr^T (PE transpose, bf16 PSUM)
            for g in range(2):
                pt = pacc.tile([128, 512], BF16, name="pt", tag="pt", bufs=2)
                for k in range(4):
                    ib = g * 4 + k
                    nc.tensor.transpose(
                        pt[:, k * 128:(k + 1) * 128],
                        hr[:, ib * 128:(ib + 1) * 128],
                        identb[:],
                    )
                nc.scalar.copy(hT[:, g * 4:(g + 1) * 4, t * 128:(t + 1) * 128], pt[:])

        # W arrives on the same (sync) ring right behind the h stream, in
        # the ob order phase 2 consumes it
        for ob in range(8):
            nc.sync.dma_start(wq_sb[:, ob, :], wq_ap[:, ob, :])

        # ---- phase 2: M[:, slice] = W_q^T G[:, slice] ----
        for gb in range(4):
            eng = nc.scalar if gb % 2 == 0 else nc.vector
            if gb % 2 == 0:
                nc.scalar.copy(gsl[:, 2 * gb:2 * gb + 2, :], gacc[gb][:])
            else:
                nc.vector.tensor_copy(gsl[:, 2 * gb:2 * gb + 2, :], gacc[gb][:])
        fillers(4)
        psm = [
            pacc.tile([128, 512], F32, name=f"psm{i}", tag="acc", bufs=4)
            for i in range(4)
        ]
        for ob in range(8):
            for ib in range(8):
                acc = psm[ib // 2]
                base = (ib % 2) * 256
                nc.tensor.matmul(
                    acc[:, base:base + 256],
                    wq_sb[:, ob, ib * 128:(ib + 1) * 128],
                    gsl[:, ob, :],
                    start=(ob == 0 and ib % 2 == 0), stop=(ob == 7),
                    skip_group_check=True,
                )
        for gb in range(4):
            if gb % 2 == 0:
                nc.scalar.copy(mq[:, 2 * gb:2 * gb + 2, :], psm[gb][:])
            else:
                nc.vector.tensor_copy(mq[:, 2 * gb:2 * gb + 2, :], psm[gb][:])
        fillers(5)

        # ---- phase 3: y[:, slice] = h @ M[:, slice] ----
        for pp in range(16):
            psy = pacc.tile([128, 512], F32, name="psy", tag="acc", bufs=4)
            for ib in range(8):
                nc.tensor.matmul(
                    psy[:, 0:256],
                    hT[:, ib, (2 * pp) * 128:(2 * pp + 1) * 128],
                    mq[:, ib, :],
                    start=(ib == 0), stop=(ib == 7),
                    skip_group_check=True,
                )
                nc.tensor.matmul(
                    psy[:, 256:512],
                    hT[:, ib, (2 * pp + 1) * 128:(2 * pp + 2) * 128],
                    mq[:, ib, :],
                    start=False, stop=(ib == 7),
                    skip_group_check=True,
                )
            ysb = stream.tile([128, 512], BF16, name="ysb", tag="yo", bufs=4)
            if pp % 2 == 0:
                nc.scalar.copy(ysb[:], psy[:])
            else:
                nc.vector.tensor_copy(ysb[:], psy[:])
            nc.scalar.dma_start(
                y_ap[:, 2 * pp:2 * pp + 2, :],
                ysb[:].rearrange("p (t c) -> p t c", t=2),
            )


_NC_CACHE = {}


def _build():
    if "nc" in _NC_CACHE:
        return _NC_CACHE["nc"]
    nc = bacc.Bacc("TRN2", target_bir_lowering=False, debug=False, num_devices=8)
    hcs_d = nc.dram_tensor("hcs", [L, 2 * H], BF16, kind="ExternalInput")
    wq_d = nc.dram_tensor("wq", [H, H], BF16, kind="ExternalInput")
    y_d = nc.dram_tensor("y", [L, NCS], BF16, kind="ExternalOutput")
    with tile.TileContext(nc) as tc:
        _emit_once(nc, tc, hcs_d, wq_d, y_d)
    nc.compile()
    _NC_CACHE["nc"] = nc
    return nc


def _prep_inputs(hidden_states, W_q, cos, sin):
    """Shard/permute on host: per core j, roll the RoPE pair axis by 128*j
    so the core's G/M/y column slice sits at permuted cols {0:128,512:640};
    W_q gets the same permutation on both axes. Pack [h|cos|sin] into one
    row-major bf16 tensor so the stream is one DMA per l-tile."""
    hs = np.asarray(hidden_states, dtype=np.float32)
    wq = np.asarray(W_q, dtype=np.float32)
    c1 = np.asarray(cos, dtype=np.float32)[:, :HH]
    s1 = np.asarray(sin, dtype=np.float32)[:, :HH]

    in_maps = []
    for core in range(8):
        b, j = core // 4, core % 4
        pr = (np.arange(HH) + PAIRS * j) % HH          # pair permutation
        idx = np.concatenate([pr, HH + pr])            # full column perm
        hp = hs[b][:, idx]
        hcs = np.concatenate([hp, c1[:, pr], s1[:, pr]], axis=1)
        wp = wq[np.ix_(idx, idx)]
        in_maps.append({
            "hcs": np.ascontiguousarray(hcs.astype(BF16_NP)),
            "wq": np.ascontiguousarray(wp.astype(BF16_NP)),
        })
    return in_maps


def kernel(hidden_states, W_q, cos, sin):
    in_maps = _prep_inputs(hidden_states, W_q, cos, sin)
    nc = _build()
    res = run_bass_kernel_spmd(nc, in_maps, core_ids=list(range(8)))

    out = np.empty((B, L, H), dtype=np.float32)
    for core, r in enumerate(res.results):
        b, j = core // 4, core % 4
        y = np.asarray(r["y"]).astype(np.float32)      # [L, 256] packed lo|hi
        out[b][:, PAIRS * j:PAIRS * (j + 1)] = y[:, 0:PAIRS]
        out[b][:, HH + PAIRS * j:HH + PAIRS * (j + 1)] = y[:, PAIRS:NCS]
    return out


# revision 21
# speedup vs baseline: 2.0558x; 1.4207x over previous
"""Trainium2 Bass kernel for nn_DenseAttentionOneHead (B=2, L=4096, H=1024).

Reference math:
    h   = hidden * cos + rotate_half(hidden) * sin      (RoPE)
    q   = h @ W_q.T
    out = (q @ h^T) @ h                                 (no softmax)

With no softmax the L x L score matrix factorizes away and W_q folds in:
    out[b] = h[b] @ M[b],  M[b] = W_q^T G[b],  G[b] = h[b]^T h[b]  (H x H)

Sharding (8 NeuronCores, ZERO collectives): cores are column-parallel
inside each batch. Core (b, j) computes a 256-column slice of G, M and y
for batch b with the FULL L=4096 contraction, reading all of h[b]. The
cost-model collectives (15us fixed + bytes/40GBps each) priced the old
row-parallel ReduceScatter+AllGather exchange at ~95us of serial time;
re-reading h (8MB bf16, ~23us of DMA overlapped under compute) is far
cheaper.

All 8 cores run one SPMD module: the host rolls the RoPE *pair* axis by
128*j so every core's slice lands at permuted columns {0:128, 512:640},
and permutes W_q on both axes to match (contractions are order-agnostic).
The host un-permutes the output columns when scattering back.

Per-core phases (PE totals 180K cycles = 75us at 2.4GHz):
  1. Stream 32 l-tiles of packed [h|cos|sin] (one 4KB-desc DMA each);
     RoPE on DVE+Pool via scalar_tensor_tensor (4x DVE mode, 194ns/op);
     accumulate G[:, slice] into 4 persistent PSUM banks; PE-transpose
     each RoPE'd tile into hT (bf16, 1 cyc/row) for phase 3; ACT drains
     the transpose banks. One 512-row PE filler per tile bridges the
     ~180ns/tile DMA-vs-PE deficit so the PE p-state never drops.
  2. M[:, slice] = W_q^T G[:, slice]: 64 matmuls, W arriving right
     behind the h stream on the same DMA ring.
  3. y[:, slice] = h @ M[:, slice]: 256 matmuls from hT against M,
     PSUM drained via DVE/ACT casts to bf16 and DMA'd out.
"""

import os

import numpy as np

import jax

try:
    _cache_dir = os.path.join(os.path.expanduser("~"), ".cache", "bass_kernel_jax")
    os.makedirs(_cache_dir, exist_ok=True)
    jax.config.update("jax_compilation_cache_dir", _cache_dir)
    jax.config.update("jax_persistent_cache_min_compile_time_secs", 1.0)
except Exception:
    pass

import ml_dtypes

import concourse.bacc as bacc
import concourse.bass as bass
import concourse.mybir as mybir
import concourse.tile as tile
from concourse import masks
from concourse.bass_utils import run_bass_kernel_spmd

F32 = mybir.dt.float32
BF16 = mybir.dt.bfloat16
BF16_NP = ml_dtypes.bfloat16

B, L, H = 2, 4096, 1024
HH = H // 2          # 512 RoPE pairs
NT = L // 128        # 32 l-tiles
NCS = 256            # columns of G/M/y per core
PAIRS = 128          # RoPE pairs per core slice

BYP = mybir.AluOpType.bypass
MUL = mybir.AluOpType.mult
ADD = mybir.AluOpType.add
SUB = mybir.AluOpType.subtract


def _emit_once(nc, tc, hcs_d, wq_d, y_d):
    # hcs rows are l; cols = [h_perm (1024) | cos_perm (512) | sin_perm (512)]
    hcs_ap = hcs_d.ap().rearrange("(t p) c -> p t c", p=128)  # [128, 32, 2048]
    wq_ap = wq_d.ap().rearrange("(t p) c -> p t c", p=128)    # [128, 8, 1024]
    y_ap = y_d.ap()                                           # yT [256, 4096]

    with (
        tc.tile_pool(name="persist", bufs=1) as persist,
        tc.tile_pool(name="stream", bufs=1) as stream,
        tc.tile_pool(name="pacc", bufs=1, space="PSUM") as pacc,
    ):
        hT = persist.tile([128, 8, L], BF16, name="hT")        # h^T, 64KB/part
        wq_sb = persist.tile([128, 8, H], BF16, name="wq_sb")  # W_q, 16KB/part
        gsl = persist.tile([128, 8, NCS], BF16, name="gsl")    # G[:, slice]
        mq = persist.tile([128, 8, NCS], BF16, name="mq")      # M[:, slice]
        dum = persist.tile([128, 512], BF16, name="dum")       # filler operand
        nc.vector.memset(dum[:], 0.0)

        identf = stream.tile([128, 128], F32, name="identf")
        masks.make_identity(nc, identf[:])
        identb = stream.tile([128, 128], BF16, name="identb")
        nc.vector.tensor_copy(identb[:], identf[:])

        fill = pacc.tile([128, 512], F32, name="fill", tag="fill", bufs=1)

        def fillers(n, rows=512):
            # no-reader self-matmuls keep the PE p-state ramped across
            # short DMA/copy waits (post-idle matmuls cost up to 2x)
            for _ in range(n):
                nc.tensor.matmul(
                    fill[:, 0:rows], dum[:, 0:128], dum[:, 0:rows],
                    start=True, stop=True, skip_group_check=True,
                )

        # G[:, slice] accumulators: 4 banks, 2 row-blocks each, packed
        # [ob_even: lo|hi, ob_odd: lo|hi] = [0:128|128:256, 256:384|384:512]
        gacc = [
            pacc.tile([128, 512], F32, name=f"gacc{i}", tag="acc", bufs=4)
            for i in range(4)
        ]

        # ---- phase 1: stream h|c|s, RoPE, G accumulation, transposes ----
        for t in range(NT):
            if t == NT - 1:
                # W rides the same ring just ahead of the last h tile: it
                # lands right as the G drain finishes, so phase 2 never
                # stalls, while tile 31 still arrives before DVE needs it
                for ob in range(8):
                    nc.sync.dma_start(wq_sb[:, ob, :], wq_ap[:, ob, :])
            hc = stream.tile([128, 2048], BF16, name="hc", tag="ld", bufs=4)
            nc.sync.dma_start(hc[:], hcs_ap[:, t, :])
            hr = stream.tile([128, H], BF16, name="hr", tag="hr", bufs=6)
            mA = stream.tile([128, 2, HH], BF16, name="mA", tag="mA", bufs=2)
            mB = stream.tile([128, 2, HH], BF16, name="mB", tag="mB", bufs=2)
            # RoPE via two 1024-wide DVE muls (amortizes the access-latency
            # bubble): mA = [h1|h2]*[c|s] = [m1|m2]; mB = [h2|h1]*[c|s]
            # (negative-stride swapped view) = [m3|m4]. Then
            # hr_lo = m1 - m2 (DVE; Pool every 4th tile), hr_hi = m3 + m4
            # (Pool) — DVE/Pool both stay under the 1456ns/tile DMA pace.
            hpair = hc[:, 0:H].rearrange("p (o c) -> p o c", o=2)
            hbase = hc[:, 0:H]
            hswap = bass.AP(
                hbase.tensor, hbase.offset + HH,
                [hbase.ap[0], [-HH, 2], [1, HH]],
            )
            cs = hc[:, H:2 * H].rearrange("p (o c) -> p o c", o=2)
            nc.vector.tensor_mul(mA[:], hpair, cs)
            nc.vector.tensor_mul(mB[:], hswap, cs)
            sub_eng = nc.gpsimd if t % 4 == 3 else nc.vector
            sub_eng.tensor_sub(hr[:, 0:HH], mA[:, 0, :], mA[:, 1, :])
            nc.gpsimd.tensor_add(hr[:, HH:H], mB[:, 0, :], mB[:, 1, :])

            if t >= 1:
                fillers(1, rows=256)
            # G[:, slice] += hr^T hr[:, slice]; slice = {0:128, 512:640} as
            # one 2-level moving AP. HW: start=True zeroes the WHOLE psum
            # bank, so only the first group touching a bank starts; the
            # co-resident group accumulates onto the zeroed bank.
            hrs = hr[:].rearrange("p (h c) -> p h c", h=2)[:, :, 0:PAIRS]
            for ob in range(8):
                acc = gacc[ob // 2]
                base = (ob % 2) * 256
                nc.tensor.matmul(
                    acc[:, base:base + 256],
                    hr[:, ob * 128:(ob + 1) * 128], hrs,
                    start=(t == 0 and ob % 2 == 0), stop=(t == NT - 1),
                    skip_group_check=True,
                )
            # hT[:, :, t*128:(t+1)*128] = hr^T (PE transpose, bf16 PSUM)
            for g in range(2):
                pt = pacc.tile([128, 512], BF16, name="pt", tag="pt", bufs=2)
                for k in range(4):
                    ib = g * 4 + k
                    nc.tensor.transpose(
                        pt[:, k * 128:(k + 1) * 128],
                        hr[:, ib * 128:(ib + 1) * 128],
                        identb[:],
                    )
                nc.scalar.copy(hT[:, g * 4:(g + 1) * 4, t * 128:(t + 1) * 128], pt[:])

        # ---- phase 2: M[:, slice] = W_q^T G[:, slice] ----
        for gb in range(4):
            eng = nc.scalar if gb % 2 == 0 else nc.vector
            if gb % 2 == 0:
                nc.scalar.copy(gsl[:, 2 * gb:2 * gb + 2, :], gacc[gb][:])
            else:
                nc.vector.tensor_copy(gsl[:, 2 * gb:2 * gb + 2, :], gacc[gb][:])
        fillers(4)
        psm = [
            pacc.tile([128, 512], F32, name=f"psm{i}", tag="acc", bufs=4)
            for i in range(4)
        ]
        for ob in range(8):
            for ib in range(8):
                acc = psm[ib // 2]
                base = (ib % 2) * 256
                nc.tensor.matmul(
                    acc[:, base:base + 256],
                    wq_sb[:, ob, ib * 128:(ib + 1) * 128],
                    gsl[:, ob, :],
                    start=(ob == 0 and ib % 2 == 0), stop=(ob == 7),
                    skip_group_check=True,
                )
            if ob < 3:
                # bridge the W-DMA / gsl-copy arrival pace
                fillers(1)
        for gb in range(4):
            if gb % 2 == 0:
                nc.scalar.copy(mq[:, 2 * gb:2 * gb + 2, :], psm[gb][:])
            else:
                nc.vector.tensor_copy(mq[:, 2 * gb:2 * gb + 2, :], psm[gb][:])
        fillers(5)

        # ---- phase 3: yT[slice, :] = M[:, slice]^T @ h^T ----
        # transposed orientation: stationary = M column block, moving = hT
        # 512-row chunks — half the matmul count of the y-major form. The
        # host transposes yT back when scattering output columns.
        for lc in range(8):
            for cb in range(2):
                psy = pacc.tile([128, 512], F32, name="psy", tag="acc", bufs=4)
                for ib in range(8):
                    nc.tensor.matmul(
                        psy[:],
                        mq[:, ib, cb * 128:(cb + 1) * 128],
                        hT[:, ib, lc * 512:(lc + 1) * 512],
                        start=(ib == 0), stop=(ib == 7),
                    )
                ysb = stream.tile([128, 512], BF16, name="ysb", tag="yo", bufs=4)
                last = lc == 7 and cb == 1
                if last:
                    # split the final drain across both engines + two DMAs
                    nc.scalar.copy(ysb[:, 0:256], psy[:, 0:256])
                    nc.vector.tensor_copy(ysb[:, 256:512], psy[:, 256:512])
                    nc.scalar.dma_start(
                        y_ap[cb * 128:(cb + 1) * 128,
                             lc * 512:lc * 512 + 256], ysb[:, 0:256])
                    nc.scalar.dma_start(
                        y_ap[cb * 128:(cb + 1) * 128,
                             lc * 512 + 256:(lc + 1) * 512], ysb[:, 256:512])
                else:
                    if (lc * 2 + cb) % 2 == 0:
                        nc.scalar.copy(ysb[:], psy[:])
                    else:
                        nc.vector.tensor_copy(ysb[:], psy[:])
                    nc.scalar.dma_start(
                        y_ap[cb * 128:(cb + 1) * 128, lc * 512:(lc + 1) * 512],
                        ysb[:],
                    )


_NC_CACHE = {}


def _build():
    if "nc" in _NC_CACHE:
        return _NC_CACHE["nc"]
    nc = bacc.Bacc("TRN2", target_bir_lowering=False, debug=False, num_devices=8)
    hcs_d = nc.dram_tensor("hcs", [L, 2 * H], BF16, kind="ExternalInput")
    wq_d = nc.dram_tensor("wq", [H, H], BF16, kind="ExternalInput")
    y_d = nc.dram_tensor("y", [NCS, L], BF16, kind="ExternalOutput")
    with tile.TileContext(nc) as tc:
        _emit_once(nc, tc, hcs_d, wq_d, y_d)
    nc.compile()
    _NC_CACHE["nc"] = nc
    return nc


def _prep_inputs(hidden_states, W_q, cos, sin):
    """Shard/permute on host: per core j, roll the RoPE pair axis by 128*j
    so the core's G/M/y column slice sits at permuted cols {0:128,512:640};
    W_q gets the same permutation on both axes. Pack [h|cos|sin] into one
    row-major bf16 tensor so the stream is one DMA per l-tile."""
    hs = np.asarray(hidden_states, dtype=np.float32)
    wq = np.asarray(W_q, dtype=np.float32)
    c1 = np.asarray(cos, dtype=np.float32)[:, :HH]
    s1 = np.asarray(sin, dtype=np.float32)[:, :HH]

    in_maps = []
    for core in range(8):
        b, j = core // 4, core % 4
        pr = (np.arange(HH) + PAIRS * j) % HH          # pair permutation
        idx = np.concatenate([pr, HH + pr])            # full column perm
        hp = hs[b][:, idx]
        hcs = np.concatenate([hp, c1[:, pr], s1[:, pr]], axis=1)
        wp = wq[np.ix_(idx, idx)]
        in_maps.append({
            "hcs": np.ascontiguousarray(hcs.astype(BF16_NP)),
            "wq": np.ascontiguousarray(wp.astype(BF16_NP)),
        })
    return in_maps


def kernel(hidden_states, W_q, cos, sin):
    in_maps = _prep_inputs(hidden_states, W_q, cos, sin)
    nc = _build()
    res = run_bass_kernel_spmd(nc, in_maps, core_ids=list(range(8)))

    out = np.empty((B, L, H), dtype=np.float32)
    for core, r in enumerate(res.results):
        b, j = core // 4, core % 4
        yt = np.asarray(r["y"]).astype(np.float32)     # [256, L] packed lo|hi
        out[b][:, PAIRS * j:PAIRS * (j + 1)] = yt[0:PAIRS].T
        out[b][:, HH + PAIRS * j:HH + PAIRS * (j + 1)] = yt[PAIRS:NCS].T
    return out


# revision 40
# speedup vs baseline: 2.1714x; 1.0562x over previous
"""Trainium2 Bass kernel for nn_DenseAttentionOneHead (B=2, L=4096, H=1024).

Reference math:
    h   = hidden * cos + rotate_half(hidden) * sin      (RoPE)
    q   = h @ W_q.T
    out = (q @ h^T) @ h                                 (no softmax)

With no softmax the L x L score matrix factorizes away and W_q folds in:
    out[b] = h[b] @ M[b],  M[b] = W_q^T G[b],  G[b] = h[b]^T h[b]  (H x H)

Sharding (8 NeuronCores, ZERO collectives): cores are column-parallel
inside each batch. Core (b, j) computes a 256-column slice of G, M and y
for batch b with the FULL L=4096 contraction, reading all of h[b]. The
cost-model collectives (15us fixed + bytes/40GBps each) priced the old
row-parallel ReduceScatter+AllGather exchange at ~95us of serial time;
re-reading h (8MB bf16, ~23us of DMA overlapped under compute) is far
cheaper.

All 8 cores run one SPMD module: the host rolls the RoPE *pair* axis by
128*j so every core's slice lands at permuted columns {0:128, 512:640},
and permutes W_q on both axes to match (contractions are order-agnostic).
The host un-permutes the output columns when scattering back.

Per-core phases (~93us simulated; phase 1 paced by the 1456ns/tile DMA):
  1. Stream 32 l-tiles of packed [h|cos|sin] (one 4KB-desc DMA each).
     RoPE as two 1024-wide DVE muls — [h1|h2]*[c|s] and a negative-stride
     swapped view [h2|h1]*[c|s] — plus a sub (DVE, Pool every 4th tile)
     and an add (Pool), keeping DVE ~1431ns and Pool ~1389ns per tile,
     both under the DMA pace. G[:, slice] accumulates into 4 persistent
     PSUM banks (2 row-blocks per bank; only the bank's first group may
     use start=True since start zeroes the whole bank). Each tile is also
     PE-transposed (bf16, 1 cyc/row) into hT with ACT draining the
     transpose banks (last tile on DVE so ACT is free for the G drain).
  2. M[:, slice] = W_q^T G[:, slice]: 64 matmuls, W arriving right
     behind the h stream on the same DMA ring. A few no-reader PE filler
     matmuls bridge the drain/DMA waits so the p-state never drops.
  3. yT[slice, :] = M[:, slice]^T hT: transposed orientation gives
     512-row moving operands (128 matmuls); PSUM drained via DVE/ACT
     casts to bf16 and DMA'd out; the host re-transposes when scattering.
"""

import os

import numpy as np

import jax

try:
    _cache_dir = os.path.join(os.path.expanduser("~"), ".cache", "bass_kernel_jax")
    os.makedirs(_cache_dir, exist_ok=True)
    jax.config.update("jax_compilation_cache_dir", _cache_dir)
    jax.config.update("jax_persistent_cache_min_compile_time_secs", 1.0)
except Exception:
    pass

import ml_dtypes

import concourse.bacc as bacc
import concourse.bass as bass
import concourse.mybir as mybir
import concourse.tile as tile
from concourse import masks
from concourse.bass_utils import run_bass_kernel_spmd

F32 = mybir.dt.float32
BF16 = mybir.dt.bfloat16
BF16_NP = ml_dtypes.bfloat16

B, L, H = 2, 4096, 1024
HH = H // 2          # 512 RoPE pairs
NT = L // 128        # 32 l-tiles
NCS = 256            # columns of G/M/y per core
PAIRS = 128          # RoPE pairs per core slice

BYP = mybir.AluOpType.bypass
MUL = mybir.AluOpType.mult
ADD = mybir.AluOpType.add
SUB = mybir.AluOpType.subtract


def _emit_once(nc, tc, hcs_d, wq_d, y_d):
    # hcs rows are l; cols = [h_perm (1024) | cos_perm (512) | sin_perm (512)]
    hcs_ap = hcs_d.ap().rearrange("(t p) c -> p t c", p=128)  # [128, 32, 2048]
    wq_ap = wq_d.ap().rearrange("(t p) c -> p t c", p=128)    # [128, 8, 1024]
    y_ap = y_d.ap()                                           # yT [256, 4096]

    with (
        tc.tile_pool(name="persist", bufs=1) as persist,
        tc.tile_pool(name="stream", bufs=1) as stream,
        tc.tile_pool(name="pacc", bufs=1, space="PSUM") as pacc,
    ):
        hT = persist.tile([128, 8, L], BF16, name="hT")        # h^T, 64KB/part
        wq_sb = persist.tile([128, 8, H], BF16, name="wq_sb")  # W_q, 16KB/part
        gsl = persist.tile([128, 8, NCS], BF16, name="gsl")    # G[:, slice]
        mq = persist.tile([128, 8, NCS], BF16, name="mq")      # M[:, slice]
        dum = persist.tile([128, 512], BF16, name="dum")       # filler operand
        nc.vector.memset(dum[:], 0.0)

        identf = stream.tile([128, 128], F32, name="identf")
        masks.make_identity(nc, identf[:])
        identb = stream.tile([128, 128], BF16, name="identb")
        nc.vector.tensor_copy(identb[:], identf[:])

        fill = pacc.tile([128, 512], F32, name="fill", tag="fill", bufs=1)

        def fillers(n, rows=512):
            # no-reader self-matmuls keep the PE p-state ramped across
            # short DMA/copy waits (post-idle matmuls cost up to 2x)
            for _ in range(n):
                nc.tensor.matmul(
                    fill[:, 0:rows], dum[:, 0:128], dum[:, 0:rows],
                    start=True, stop=True, skip_group_check=True,
                )

        # G[:, slice] accumulators: 4 banks, 2 row-blocks each, packed
        # [ob_even: lo|hi, ob_odd: lo|hi] = [0:128|128:256, 256:384|384:512]
        gacc = [
            pacc.tile([128, 512], F32, name=f"gacc{i}", tag="acc", bufs=4)
            for i in range(4)
        ]

        # ---- phase 1: stream h|c|s, RoPE, G accumulation, transposes ----
        for t in range(NT):
            hc = stream.tile([128, 2048], BF16, name="hc", tag="ld", bufs=6)
            nc.sync.dma_start(hc[:], hcs_ap[:, t, :])
            hr = stream.tile([128, H], BF16, name="hr", tag="hr", bufs=6)
            mA = stream.tile([128, 2, HH], BF16, name="mA", tag="mA", bufs=2)
            mB = stream.tile([128, 2, HH], BF16, name="mB", tag="mB", bufs=2)
            # RoPE via two 1024-wide DVE muls (amortizes the access-latency
            # bubble): mA = [h1|h2]*[c|s] = [m1|m2]; mB = [h2|h1]*[c|s]
            # (negative-stride swapped view) = [m3|m4]. Then
            # hr_lo = m1 - m2 (DVE; Pool every 4th tile), hr_hi = m3 + m4
            # (Pool) — DVE/Pool both stay under the 1456ns/tile DMA pace.
            hpair = hc[:, 0:H].rearrange("p (o c) -> p o c", o=2)
            hbase = hc[:, 0:H]
            hswap = bass.AP(
                hbase.tensor, hbase.offset + HH,
                [hbase.ap[0], [-HH, 2], [1, HH]],
            )
            cs = hc[:, H:2 * H].rearrange("p (o c) -> p o c", o=2)
            nc.vector.tensor_mul(mA[:], hpair, cs)
            nc.vector.tensor_mul(mB[:], hswap, cs)
            sub_eng = nc.gpsimd if t % 4 == 3 else nc.vector
            sub_eng.tensor_sub(hr[:, 0:HH], mA[:, 0, :], mA[:, 1, :])
            nc.gpsimd.tensor_add(hr[:, HH:H], mB[:, 0, :], mB[:, 1, :])

            # G[:, slice] += hr^T hr[:, slice]; slice = {0:128, 512:640} as
            # one 2-level moving AP. HW: start=True zeroes the WHOLE psum
            # bank, so only the first group touching a bank starts; the
            # co-resident group accumulates onto the zeroed bank.
            hrs = hr[:].rearrange("p (h c) -> p h c", h=2)[:, :, 0:PAIRS]
            for ob in range(8):
                acc = gacc[ob // 2]
                base = (ob % 2) * 256
                nc.tensor.matmul(
                    acc[:, base:base + 256],
                    hr[:, ob * 128:(ob + 1) * 128], hrs,
                    start=(t == 0 and ob % 2 == 0), stop=(t == NT - 1),
                    skip_group_check=True,
                )
            # hT[:, :, t*128:(t+1)*128] = hr^T (PE transpose, bf16 PSUM).
            # The last tile's drains go to Pool so ACT's queue is clear for
            # the G drain the moment the accumulators stop.
            for g in range(2):
                pt = pacc.tile([128, 512], BF16, name="pt", tag="pt", bufs=2)
                for k in range(4):
                    ib = g * 4 + k
                    nc.tensor.transpose(
                        pt[:, k * 128:(k + 1) * 128],
                        hr[:, ib * 128:(ib + 1) * 128],
                        identb[:],
                    )
                dst = hT[:, g * 4:(g + 1) * 4, t * 128:(t + 1) * 128]
                if t == NT - 1:
                    nc.vector.tensor_copy(dst, pt[:])
                else:
                    nc.scalar.copy(dst, pt[:])

        # W arrives on the same (sync) ring right behind the h stream, in
        # the ob order phase 2 consumes it
        for ob in range(8):
            nc.sync.dma_start(wq_sb[:, ob, :], wq_ap[:, ob, :])

        # ---- phase 2: M[:, slice] = W_q^T G[:, slice] ----
        for gb in range(4):
            eng = nc.scalar if gb % 2 == 0 else nc.vector
            if gb % 2 == 0:
                nc.scalar.copy(gsl[:, 2 * gb:2 * gb + 2, :], gacc[gb][:])
            else:
                nc.vector.tensor_copy(gsl[:, 2 * gb:2 * gb + 2, :], gacc[gb][:])
        fillers(3, rows=256)
        psm = [
            pacc.tile([128, 512], F32, name=f"psm{i}", tag="acc", bufs=4)
            for i in range(4)
        ]
        for ob in range(8):
            for ib in range(8):
                acc = psm[ib // 2]
                base = (ib % 2) * 256
                nc.tensor.matmul(
                    acc[:, base:base + 256],
                    wq_sb[:, ob, ib * 128:(ib + 1) * 128],
                    gsl[:, ob, :],
                    start=(ob == 0 and ib % 2 == 0), stop=(ob == 7),
                    skip_group_check=True,
                )
            if ob < 3:
                # bridge the W-DMA / gsl-copy arrival pace
                fillers(1, rows=256)
        for gb in range(4):
            if gb % 2 == 0:
                nc.scalar.copy(mq[:, 2 * gb:2 * gb + 2, :], psm[gb][:])
            else:
                nc.vector.tensor_copy(mq[:, 2 * gb:2 * gb + 2, :], psm[gb][:])

        fillers(3, rows=256)

        # ---- phase 3: yT[slice, :] = M[:, slice]^T @ h^T ----
        # transposed orientation: stationary = M column block, moving = hT
        # 512-row chunks — half the matmul count of the y-major form. The
        # host transposes yT back when scattering output columns.
        for lc in range(8):
            for cb in range(2):
                psy = pacc.tile([128, 512], F32, name="psy", tag="acc", bufs=4)
                for ib in range(8):
                    nc.tensor.matmul(
                        psy[:],
                        mq[:, ib, cb * 128:(cb + 1) * 128],
                        hT[:, ib, lc * 512:(lc + 1) * 512],
                        start=(ib == 0), stop=(ib == 7),
                    )
                ysb = stream.tile([128, 512], BF16, name="ysb", tag="yo", bufs=4)
                last = lc == 7 and cb == 1
                if last:
                    # split the final drain across both engines + two DMAs
                    # on separate SEQ rings so the issue chains overlap
                    nc.scalar.copy(ysb[:, 0:256], psy[:, 0:256])
                    nc.vector.tensor_copy(ysb[:, 256:512], psy[:, 256:512])
                    nc.scalar.dma_start(
                        y_ap[cb * 128:(cb + 1) * 128,
                             lc * 512:lc * 512 + 256], ysb[:, 0:256])
                    nc.sync.dma_start(
                        y_ap[cb * 128:(cb + 1) * 128,
                             lc * 512 + 256:(lc + 1) * 512], ysb[:, 256:512])
                else:
                    if (lc * 2 + cb) % 2 == 0:
                        nc.scalar.copy(ysb[:], psy[:])
                    else:
                        nc.vector.tensor_copy(ysb[:], psy[:])
                    nc.scalar.dma_start(
                        y_ap[cb * 128:(cb + 1) * 128, lc * 512:(lc + 1) * 512],
                        ysb[:],
                    )


_NC_CACHE = {}


def _build():
    if "nc" in _NC_CACHE:
        return _NC_CACHE["nc"]
    nc = bacc.Bacc("TRN2", target_bir_lowering=False, debug=False, num_devices=8)
    hcs_d = nc.dram_tensor("hcs", [L, 2 * H], BF16, kind="ExternalInput")
    wq_d = nc.dram_tensor("wq", [H, H], BF16, kind="ExternalInput")
    y_d = nc.dram_tensor("y", [NCS, L], BF16, kind="ExternalOutput")
    with tile.TileContext(nc) as tc:
        _emit_once(nc, tc, hcs_d, wq_d, y_d)
    nc.compile()
    _NC_CACHE["nc"] = nc
    return nc


def _prep_inputs(hidden_states, W_q, cos, sin):
    """Shard/permute on host: per core j, roll the RoPE pair axis by 128*j
    so the core's G/M/y column slice sits at permuted cols {0:128,512:640};
    W_q gets the same permutation on both axes. Pack [h|cos|sin] into one
    row-major bf16 tensor so the stream is one DMA per l-tile."""
    hs = np.asarray(hidden_states, dtype=np.float32)
    wq = np.asarray(W_q, dtype=np.float32)
    c1 = np.asarray(cos, dtype=np.float32)[:, :HH]
    s1 = np.asarray(sin, dtype=np.float32)[:, :HH]

    in_maps = []
    for core in range(8):
        b, j = core // 4, core % 4
        pr = (np.arange(HH) + PAIRS * j) % HH          # pair permutation
        idx = np.concatenate([pr, HH + pr])            # full column perm
        hp = hs[b][:, idx]
        hcs = np.concatenate([hp, c1[:, pr], s1[:, pr]], axis=1)
        wp = wq[np.ix_(idx, idx)]
        in_maps.append({
            "hcs": np.ascontiguousarray(hcs.astype(BF16_NP)),
            "wq": np.ascontiguousarray(wp.astype(BF16_NP)),
        })
    return in_maps


def kernel(hidden_states, W_q, cos, sin):
    in_maps = _prep_inputs(hidden_states, W_q, cos, sin)
    nc = _build()
    res = run_bass_kernel_spmd(nc, in_maps, core_ids=list(range(8)))

    out = np.empty((B, L, H), dtype=np.float32)
    for core, r in enumerate(res.results):
        b, j = core // 4, core % 4
        yt = np.asarray(r["y"]).astype(np.float32)     # [256, L] packed lo|hi
        out[b][:, PAIRS * j:PAIRS * (j + 1)] = yt[0:PAIRS].T
        out[b][:, HH + PAIRS * j:HH + PAIRS * (j + 1)] = yt[PAIRS:NCS].T
    return out


# revision 44
# speedup vs baseline: 2.1787x; 1.0034x over previous
"""Trainium2 Bass kernel for nn_DenseAttentionOneHead (B=2, L=4096, H=1024).

Reference math:
    h   = hidden * cos + rotate_half(hidden) * sin      (RoPE)
    q   = h @ W_q.T
    out = (q @ h^T) @ h                                 (no softmax)

With no softmax the L x L score matrix factorizes away and W_q folds in:
    out[b] = h[b] @ M[b],  M[b] = W_q^T G[b],  G[b] = h[b]^T h[b]  (H x H)

Sharding (8 NeuronCores, ZERO collectives): cores are column-parallel
inside each batch. Core (b, j) computes a 256-column slice of G, M and y
for batch b with the FULL L=4096 contraction, reading all of h[b]. The
cost-model collectives (15us fixed + bytes/40GBps each) priced the old
row-parallel ReduceScatter+AllGather exchange at ~95us of serial time;
re-reading h (8MB bf16, ~23us of DMA overlapped under compute) is far
cheaper.

All 8 cores run one SPMD module: the host rolls the RoPE *pair* axis by
128*j so every core's slice lands at permuted columns {0:128, 512:640},
and permutes W_q on both axes to match (contractions are order-agnostic).
The host un-permutes the output columns when scattering back.

Per-core phases (~93us simulated; phase 1 paced by the 1456ns/tile DMA):
  1. Stream 32 l-tiles of packed [h|cos|sin] (one 4KB-desc DMA each).
     RoPE as two 1024-wide DVE muls — [h1|h2]*[c|s] and a negative-stride
     swapped view [h2|h1]*[c|s] — plus a sub (DVE, Pool every 4th tile)
     and an add (Pool), keeping DVE ~1431ns and Pool ~1389ns per tile,
     both under the DMA pace. G[:, slice] accumulates into 4 persistent
     PSUM banks (2 row-blocks per bank; only the bank's first group may
     use start=True since start zeroes the whole bank). Each tile is also
     PE-transposed (bf16, 1 cyc/row) into hT with ACT draining the
     transpose banks (last tile on DVE so ACT is free for the G drain).
  2. M[:, slice] = W_q^T G[:, slice]: 64 matmuls, W arriving right
     behind the h stream on the same DMA ring. A few no-reader PE filler
     matmuls bridge the drain/DMA waits so the p-state never drops.
  3. yT[slice, :] = M[:, slice]^T hT: transposed orientation gives
     512-row moving operands (128 matmuls); PSUM drained via DVE/ACT
     casts to bf16 and DMA'd out; the host re-transposes when scattering.
"""

import os

import numpy as np

import jax

try:
    _cache_dir = os.path.join(os.path.expanduser("~"), ".cache", "bass_kernel_jax")
    os.makedirs(_cache_dir, exist_ok=True)
    jax.config.update("jax_compilation_cache_dir", _cache_dir)
    jax.config.update("jax_persistent_cache_min_compile_time_secs", 1.0)
except Exception:
    pass

import ml_dtypes

import concourse.bacc as bacc
import concourse.bass as bass
import concourse.mybir as mybir
import concourse.tile as tile
from concourse import masks
from concourse.bass_utils import run_bass_kernel_spmd

F32 = mybir.dt.float32
BF16 = mybir.dt.bfloat16
BF16_NP = ml_dtypes.bfloat16

B, L, H = 2, 4096, 1024
HH = H // 2          # 512 RoPE pairs
NT = L // 128        # 32 l-tiles
NCS = 256            # columns of G/M/y per core
PAIRS = 128          # RoPE pairs per core slice

BYP = mybir.AluOpType.bypass
MUL = mybir.AluOpType.mult
ADD = mybir.AluOpType.add
SUB = mybir.AluOpType.subtract


def _emit_once(nc, tc, hcs_d, wq_d, y_d):
    # hcs rows are l; cols = [h_perm (1024) | cos_perm (512) | sin_perm (512)]
    hcs_ap = hcs_d.ap().rearrange("(t p) c -> p t c", p=128)  # [128, 32, 2048]
    wq_ap = wq_d.ap().rearrange("(t p) c -> p t c", p=128)    # [128, 8, 1024]
    y_ap = y_d.ap()                                           # yT [256, 4096]

    with (
        tc.tile_pool(name="persist", bufs=1) as persist,
        tc.tile_pool(name="stream", bufs=1) as stream,
        tc.tile_pool(name="pacc", bufs=1, space="PSUM") as pacc,
    ):
        hT = persist.tile([128, 8, L], BF16, name="hT")        # h^T, 64KB/part
        wq_sb = persist.tile([128, 8, H], BF16, name="wq_sb")  # W_q, 16KB/part
        gsl = persist.tile([128, 8, NCS], BF16, name="gsl")    # G[:, slice]
        mq = persist.tile([128, 8, NCS], BF16, name="mq")      # M[:, slice]
        dum = persist.tile([128, 512], BF16, name="dum")       # filler operand
        nc.vector.memset(dum[:], 0.0)

        identf = stream.tile([128, 128], F32, name="identf")
        masks.make_identity(nc, identf[:])
        identb = stream.tile([128, 128], BF16, name="identb")
        nc.vector.tensor_copy(identb[:], identf[:])

        fill = pacc.tile([128, 512], F32, name="fill", tag="fill", bufs=1)

        def fillers(n, rows=512):
            # no-reader self-matmuls keep the PE p-state ramped across
            # short DMA/copy waits (post-idle matmuls cost up to 2x)
            for _ in range(n):
                nc.tensor.matmul(
                    fill[:, 0:rows], dum[:, 0:128], dum[:, 0:rows],
                    start=True, stop=True, skip_group_check=True,
                )

        # G[:, slice] accumulators: 4 banks, 2 row-blocks each, packed
        # [ob_even: lo|hi, ob_odd: lo|hi] = [0:128|128:256, 256:384|384:512]
        gacc = [
            pacc.tile([128, 512], F32, name=f"gacc{i}", tag="acc", bufs=4)
            for i in range(4)
        ]

        # ---- phase 1: stream h|c|s, RoPE, G accumulation, transposes ----
        for t in range(NT):
            hc = stream.tile([128, 2048], BF16, name="hc", tag="ld", bufs=6)
            nc.sync.dma_start(hc[:], hcs_ap[:, t, :])
            hr = stream.tile([128, H], BF16, name="hr", tag="hr", bufs=6)
            mA = stream.tile([128, 2, HH], BF16, name="mA", tag="mA", bufs=2)
            mB = stream.tile([128, 2, HH], BF16, name="mB", tag="mB", bufs=2)
            # RoPE via two 1024-wide DVE muls (amortizes the access-latency
            # bubble): mA = [h1|h2]*[c|s] = [m1|m2]; mB = [h2|h1]*[c|s]
            # (negative-stride swapped view) = [m3|m4]. Then
            # hr_lo = m1 - m2 (DVE; Pool every 4th tile), hr_hi = m3 + m4
            # (Pool) — DVE/Pool both stay under the 1456ns/tile DMA pace.
            hpair = hc[:, 0:H].rearrange("p (o c) -> p o c", o=2)
            hbase = hc[:, 0:H]
            hswap = bass.AP(
                hbase.tensor, hbase.offset + HH,
                [hbase.ap[0], [-HH, 2], [1, HH]],
            )
            cs = hc[:, H:2 * H].rearrange("p (o c) -> p o c", o=2)
            nc.vector.tensor_mul(mA[:], hpair, cs)
            nc.vector.tensor_mul(mB[:], hswap, cs)
            sub_eng = nc.gpsimd if t % 4 == 3 else nc.vector
            sub_eng.tensor_sub(hr[:, 0:HH], mA[:, 0, :], mA[:, 1, :])
            nc.gpsimd.tensor_add(hr[:, HH:H], mB[:, 0, :], mB[:, 1, :])

            # G[:, slice] += hr^T hr[:, slice]; slice = {0:128, 512:640} as
            # one 2-level moving AP. HW: start=True zeroes the WHOLE psum
            # bank, so only the first group touching a bank starts; the
            # co-resident group accumulates onto the zeroed bank.
            hrs = hr[:].rearrange("p (h c) -> p h c", h=2)[:, :, 0:PAIRS]
            for ob in range(8):
                acc = gacc[ob // 2]
                base = (ob % 2) * 256
                nc.tensor.matmul(
                    acc[:, base:base + 256],
                    hr[:, ob * 128:(ob + 1) * 128], hrs,
                    start=(t == 0 and ob % 2 == 0), stop=(t == NT - 1),
                    skip_group_check=True,
                )
            # hT[:, :, t*128:(t+1)*128] = hr^T (PE transpose, bf16 PSUM).
            # The last tile's drains go to Pool so ACT's queue is clear for
            # the G drain the moment the accumulators stop.
            for g in range(2):
                pt = pacc.tile([128, 512], BF16, name="pt", tag="pt", bufs=2)
                for k in range(4):
                    ib = g * 4 + k
                    nc.tensor.transpose(
                        pt[:, k * 128:(k + 1) * 128],
                        hr[:, ib * 128:(ib + 1) * 128],
                        identb[:],
                    )
                dst = hT[:, g * 4:(g + 1) * 4, t * 128:(t + 1) * 128]
                if t == NT - 1:
                    nc.vector.tensor_copy(dst, pt[:])
                else:
                    nc.scalar.copy(dst, pt[:])

        # W arrives on the same (sync) ring right behind the h stream, in
        # the ob order phase 2 consumes it
        for ob in range(8):
            nc.sync.dma_start(wq_sb[:, ob, :], wq_ap[:, ob, :])

        # ---- phase 2: M[:, slice] = W_q^T G[:, slice] ----
        for gb in range(4):
            eng = nc.scalar if gb % 2 == 0 else nc.vector
            if gb % 2 == 0:
                nc.scalar.copy(gsl[:, 2 * gb:2 * gb + 2, :], gacc[gb][:])
            else:
                nc.vector.tensor_copy(gsl[:, 2 * gb:2 * gb + 2, :], gacc[gb][:])
        fillers(3, rows=256)
        psm = [
            pacc.tile([128, 512], F32, name=f"psm{i}", tag="acc", bufs=4)
            for i in range(4)
        ]
        for k in range(4):
            # pair-outer so psm[k] stops early and its mq drain overlaps the
            # remaining M matmuls — y then starts right behind the last one
            for ob in range(8):
                for half in range(2):
                    ib = 2 * k + half
                    nc.tensor.matmul(
                        psm[k][:, half * 256:half * 256 + 256],
                        wq_sb[:, ob, ib * 128:(ib + 1) * 128],
                        gsl[:, ob, :],
                        start=(ob == 0 and half == 0), stop=(ob == 7),
                        skip_group_check=True,
                    )
            if k == 0:
                fillers(1, rows=256)
        for gb in range(4):
            if gb % 2 == 0:
                nc.scalar.copy(mq[:, 2 * gb:2 * gb + 2, :], psm[gb][:])
            else:
                nc.vector.tensor_copy(mq[:, 2 * gb:2 * gb + 2, :], psm[gb][:])

        fillers(3, rows=256)

        # ---- phase 3: yT[slice, :] = M[:, slice]^T @ h^T ----
        # transposed orientation: stationary = M column block, moving = hT
        # 512-row chunks — half the matmul count of the y-major form. The
        # host transposes yT back when scattering output columns.
        for lc in range(8):
            for cb in range(2):
                psy = pacc.tile([128, 512], F32, name="psy", tag="acc", bufs=4)
                for ib in range(8):
                    nc.tensor.matmul(
                        psy[:],
                        mq[:, ib, cb * 128:(cb + 1) * 128],
                        hT[:, ib, lc * 512:(lc + 1) * 512],
                        start=(ib == 0), stop=(ib == 7),
                    )
                ysb = stream.tile([128, 512], BF16, name="ysb", tag="yo", bufs=4)
                last = lc == 7 and cb == 1
                if last:
                    # split the final drain across both engines + two DMAs
                    # on separate SEQ rings so the issue chains overlap
                    nc.scalar.copy(ysb[:, 0:256], psy[:, 0:256])
                    nc.vector.tensor_copy(ysb[:, 256:512], psy[:, 256:512])
                    nc.scalar.dma_start(
                        y_ap[cb * 128:(cb + 1) * 128,
                             lc * 512:lc * 512 + 256], ysb[:, 0:256])
                    nc.sync.dma_start(
                        y_ap[cb * 128:(cb + 1) * 128,
                             lc * 512 + 256:(lc + 1) * 512], ysb[:, 256:512])
                else:
                    if (lc * 2 + cb) % 2 == 0:
                        nc.scalar.copy(ysb[:], psy[:])
                    else:
                        nc.vector.tensor_copy(ysb[:], psy[:])
                    nc.scalar.dma_start(
                        y_ap[cb * 128:(cb + 1) * 128, lc * 512:(lc + 1) * 512],
                        ysb[:],
                    )


_NC_CACHE = {}


def _build():
    if "nc" in _NC_CACHE:
        return _NC_CACHE["nc"]
    nc = bacc.Bacc("TRN2", target_bir_lowering=False, debug=False, num_devices=8)
    hcs_d = nc.dram_tensor("hcs", [L, 2 * H], BF16, kind="ExternalInput")
    wq_d = nc.dram_tensor("wq", [H, H], BF16, kind="ExternalInput")
    y_d = nc.dram_tensor("y", [NCS, L], BF16, kind="ExternalOutput")
    with tile.TileContext(nc) as tc:
        _emit_once(nc, tc, hcs_d, wq_d, y_d)
    nc.compile()
    _NC_CACHE["nc"] = nc
    return nc


def _prep_inputs(hidden_states, W_q, cos, sin):
    """Shard/permute on host: per core j, roll the RoPE pair axis by 128*j
    so the core's G/M/y column slice sits at permuted cols {0:128,512:640};
    W_q gets the same permutation on both axes. Pack [h|cos|sin] into one
    row-major bf16 tensor so the stream is one DMA per l-tile."""
    hs = np.asarray(hidden_states, dtype=np.float32)
    wq = np.asarray(W_q, dtype=np.float32)
    c1 = np.asarray(cos, dtype=np.float32)[:, :HH]
    s1 = np.asarray(sin, dtype=np.float32)[:, :HH]

    in_maps = []
    for core in range(8):
        b, j = core // 4, core % 4
        pr = (np.arange(HH) + PAIRS * j) % HH          # pair permutation
        idx = np.concatenate([pr, HH + pr])            # full column perm
        hp = hs[b][:, idx]
        hcs = np.concatenate([hp, c1[:, pr], s1[:, pr]], axis=1)
        wp = wq[np.ix_(idx, idx)]
        in_maps.append({
            "hcs": np.ascontiguousarray(hcs.astype(BF16_NP)),
            "wq": np.ascontiguousarray(wp.astype(BF16_NP)),
        })
    return in_maps


def kernel(hidden_states, W_q, cos, sin):
    in_maps = _prep_inputs(hidden_states, W_q, cos, sin)
    nc = _build()
    res = run_bass_kernel_spmd(nc, in_maps, core_ids=list(range(8)))

    out = np.empty((B, L, H), dtype=np.float32)
    for core, r in enumerate(res.results):
        b, j = core // 4, core % 4
        yt = np.asarray(r["y"]).astype(np.float32)     # [256, L] packed lo|hi
        out[b][:, PAIRS * j:PAIRS * (j + 1)] = yt[0:PAIRS].T
        out[b][:, HH + PAIRS * j:HH + PAIRS * (j + 1)] = yt[PAIRS:NCS].T
    return out


# revision 46
# speedup vs baseline: 2.1894x; 1.0049x over previous
"""Trainium2 Bass kernel for nn_DenseAttentionOneHead (B=2, L=4096, H=1024).

Reference math:
    h   = hidden * cos + rotate_half(hidden) * sin      (RoPE)
    q   = h @ W_q.T
    out = (q @ h^T) @ h                                 (no softmax)

With no softmax the L x L score matrix factorizes away and W_q folds in:
    out[b] = h[b] @ M[b],  M[b] = W_q^T G[b],  G[b] = h[b]^T h[b]  (H x H)

Sharding (8 NeuronCores, ZERO collectives): cores are column-parallel
inside each batch. Core (b, j) computes a 256-column slice of G, M and y
for batch b with the FULL L=4096 contraction, reading all of h[b]. The
cost-model collectives (15us fixed + bytes/40GBps each) priced the old
row-parallel ReduceScatter+AllGather exchange at ~95us of serial time;
re-reading h (8MB bf16, ~23us of DMA overlapped under compute) is far
cheaper.

All 8 cores run one SPMD module: the host rolls the RoPE *pair* axis by
128*j so every core's slice lands at permuted columns {0:128, 512:640},
and permutes W_q on both axes to match (contractions are order-agnostic).
The host un-permutes the output columns when scattering back.

Per-core phases (~93us simulated; phase 1 paced by the 1456ns/tile DMA):
  1. Stream 32 l-tiles of packed [h|cos|sin] (one 4KB-desc DMA each).
     RoPE as two 1024-wide DVE muls — [h1|h2]*[c|s] and a negative-stride
     swapped view [h2|h1]*[c|s] — plus a sub (DVE, Pool every 4th tile)
     and an add (Pool), keeping DVE ~1431ns and Pool ~1389ns per tile,
     both under the DMA pace. G[:, slice] accumulates into 4 persistent
     PSUM banks (2 row-blocks per bank; only the bank's first group may
     use start=True since start zeroes the whole bank). Each tile is also
     PE-transposed (bf16, 1 cyc/row) into hT with ACT draining the
     transpose banks (last tile on DVE so ACT is free for the G drain).
  2. M[:, slice] = W_q^T G[:, slice]: 64 matmuls, W arriving right
     behind the h stream on the same DMA ring. A few no-reader PE filler
     matmuls bridge the drain/DMA waits so the p-state never drops.
  3. yT[slice, :] = M[:, slice]^T hT: transposed orientation gives
     512-row moving operands (128 matmuls); PSUM drained via DVE/ACT
     casts to bf16 and DMA'd out; the host re-transposes when scattering.
"""

import os

import numpy as np

import jax

try:
    _cache_dir = os.path.join(os.path.expanduser("~"), ".cache", "bass_kernel_jax")
    os.makedirs(_cache_dir, exist_ok=True)
    jax.config.update("jax_compilation_cache_dir", _cache_dir)
    jax.config.update("jax_persistent_cache_min_compile_time_secs", 1.0)
except Exception:
    pass

import ml_dtypes

import concourse.bacc as bacc
import concourse.bass as bass
import concourse.mybir as mybir
import concourse.tile as tile
from concourse import masks
from concourse.bass_utils import run_bass_kernel_spmd

F32 = mybir.dt.float32
BF16 = mybir.dt.bfloat16
BF16_NP = ml_dtypes.bfloat16

B, L, H = 2, 4096, 1024
HH = H // 2          # 512 RoPE pairs
NT = L // 128        # 32 l-tiles
NCS = 256            # columns of G/M/y per core
PAIRS = 128          # RoPE pairs per core slice

BYP = mybir.AluOpType.bypass
MUL = mybir.AluOpType.mult
ADD = mybir.AluOpType.add
SUB = mybir.AluOpType.subtract


def _emit_once(nc, tc, hcs_d, wq_d, y_d):
    # hcs rows are l; cols = [h_perm (1024) | cos_perm (512) | sin_perm (512)]
    hcs_ap = hcs_d.ap().rearrange("(t p) c -> p t c", p=128)  # [128, 32, 2048]
    wq_ap = wq_d.ap().rearrange("(t p) c -> p t c", p=128)    # [128, 8, 1024]
    y_ap = y_d.ap()                                           # yT [256, 4096]

    with (
        tc.tile_pool(name="persist", bufs=1) as persist,
        tc.tile_pool(name="stream", bufs=1) as stream,
        tc.tile_pool(name="pacc", bufs=1, space="PSUM") as pacc,
    ):
        hT = persist.tile([128, 8, L], BF16, name="hT")        # h^T, 64KB/part
        wq_sb = persist.tile([128, 8, H], BF16, name="wq_sb")  # W_q, 16KB/part
        gsl = persist.tile([128, 8, NCS], BF16, name="gsl")    # G[:, slice]
        mq = persist.tile([128, 8, NCS], BF16, name="mq")      # M[:, slice]
        dum = persist.tile([128, 512], BF16, name="dum")       # filler operand
        nc.vector.memset(dum[:], 0.0)

        identf = stream.tile([128, 128], F32, name="identf")
        masks.make_identity(nc, identf[:])
        identb = stream.tile([128, 128], BF16, name="identb")
        nc.vector.tensor_copy(identb[:], identf[:])

        fill = pacc.tile([128, 512], F32, name="fill", tag="fill", bufs=1)

        def fillers(n, rows=512):
            # no-reader self-matmuls keep the PE p-state ramped across
            # short DMA/copy waits (post-idle matmuls cost up to 2x)
            for _ in range(n):
                nc.tensor.matmul(
                    fill[:, 0:rows], dum[:, 0:128], dum[:, 0:rows],
                    start=True, stop=True, skip_group_check=True,
                )

        # G[:, slice] accumulators: 4 banks, 2 row-blocks each, packed
        # [ob_even: lo|hi, ob_odd: lo|hi] = [0:128|128:256, 256:384|384:512]
        gacc = [
            pacc.tile([128, 512], F32, name=f"gacc{i}", tag="acc", bufs=4)
            for i in range(4)
        ]

        deferred_ht = []

        # ---- phase 1: stream h|c|s, RoPE, G accumulation, transposes ----
        for t in range(NT):
            hc = stream.tile([128, 2048], BF16, name="hc", tag="ld", bufs=6)
            nc.sync.dma_start(hc[:], hcs_ap[:, t, :])
            hr = stream.tile([128, H], BF16, name="hr", tag="hr", bufs=6)
            mA = stream.tile([128, 2, HH], BF16, name="mA", tag="mA", bufs=2)
            mB = stream.tile([128, 2, HH], BF16, name="mB", tag="mB", bufs=2)
            # RoPE via two 1024-wide DVE muls (amortizes the access-latency
            # bubble): mA = [h1|h2]*[c|s] = [m1|m2]; mB = [h2|h1]*[c|s]
            # (negative-stride swapped view) = [m3|m4]. Then
            # hr_lo = m1 - m2 (DVE; Pool every 4th tile), hr_hi = m3 + m4
            # (Pool) — DVE/Pool both stay under the 1456ns/tile DMA pace.
            hpair = hc[:, 0:H].rearrange("p (o c) -> p o c", o=2)
            hbase = hc[:, 0:H]
            hswap = bass.AP(
                hbase.tensor, hbase.offset + HH,
                [hbase.ap[0], [-HH, 2], [1, HH]],
            )
            cs = hc[:, H:2 * H].rearrange("p (o c) -> p o c", o=2)
            nc.vector.tensor_mul(mA[:], hpair, cs)
            nc.vector.tensor_mul(mB[:], hswap, cs)
            # steady state: sub mostly DVE, add on Pool (balanced just
            # under the DMA pace). For the last tiles the DMA has finished,
            # so everything moves to DVE to drain the backlog fastest.
            sub_eng = nc.gpsimd if (t % 4 == 3 and t < 28) else nc.vector
            add_eng = nc.vector if t >= 30 else nc.gpsimd
            sub_eng.tensor_sub(hr[:, 0:HH], mA[:, 0, :], mA[:, 1, :])
            add_eng.tensor_add(hr[:, HH:H], mB[:, 0, :], mB[:, 1, :])

            # G[:, slice] += hr^T hr[:, slice]; slice = {0:128, 512:640} as
            # one 2-level moving AP. HW: start=True zeroes the WHOLE psum
            # bank, so only the first group touching a bank starts; the
            # co-resident group accumulates onto the zeroed bank.
            hrs = hr[:].rearrange("p (h c) -> p h c", h=2)[:, :, 0:PAIRS]
            for ob in range(8):
                acc = gacc[ob // 2]
                base = (ob % 2) * 256
                nc.tensor.matmul(
                    acc[:, base:base + 256],
                    hr[:, ob * 128:(ob + 1) * 128], hrs,
                    start=(t == 0 and ob % 2 == 0), stop=(t == NT - 1),
                    skip_group_check=True,
                )
            # hT[:, :, t*128:(t+1)*128] = hr^T (PE transpose, bf16 PSUM).
            # The last tile's drains go to Pool so ACT's queue is clear for
            # the G drain the moment the accumulators stop.
            for g in range(2):
                pt = pacc.tile([128, 512], BF16, name="pt", tag="pt", bufs=3)
                for k in range(4):
                    ib = g * 4 + k
                    nc.tensor.transpose(
                        pt[:, k * 128:(k + 1) * 128],
                        hr[:, ib * 128:(ib + 1) * 128],
                        identb[:],
                    )
                dst = hT[:, g * 4:(g + 1) * 4, t * 128:(t + 1) * 128]
                if t >= NT - 1:
                    deferred_ht.append((dst, pt))
                else:
                    nc.scalar.copy(dst, pt[:])

        # W arrives on the same (sync) ring right behind the h stream, in
        # the ob order phase 2 consumes it
        for ob in range(8):
            nc.sync.dma_start(wq_sb[:, ob, :], wq_ap[:, ob, :])

        # ---- phase 2: M[:, slice] = W_q^T G[:, slice] ----
        for gb in range(4):
            if gb < 2:
                nc.vector.tensor_copy(gsl[:, 2 * gb:2 * gb + 2, :], gacc[gb][:])
            else:
                nc.scalar.copy(gsl[:, 2 * gb:2 * gb + 2, :], gacc[gb][:])
        for i, (dst, pt) in enumerate(deferred_ht):
            if i % 2 == 0:
                nc.scalar.copy(dst, pt[:])
            else:
                nc.vector.tensor_copy(dst, pt[:])
        fillers(3, rows=256)
        psm = [
            pacc.tile([128, 512], F32, name=f"psm{i}", tag="acc", bufs=4)
            for i in range(4)
        ]
        for ob in range(8):
            for ib in range(8):
                acc = psm[ib // 2]
                base = (ib % 2) * 256
                nc.tensor.matmul(
                    acc[:, base:base + 256],
                    wq_sb[:, ob, ib * 128:(ib + 1) * 128],
                    gsl[:, ob, :],
                    start=(ob == 0 and ib % 2 == 0), stop=(ob == 7),
                    skip_group_check=True,
                )
            if ob < 2:
                fillers(1, rows=256)
        for gb in range(4):
            if gb % 2 == 0:
                nc.scalar.copy(mq[:, 2 * gb:2 * gb + 2, :], psm[gb][:])
            else:
                nc.vector.tensor_copy(mq[:, 2 * gb:2 * gb + 2, :], psm[gb][:])

        fillers(3, rows=256)

        # ---- phase 3: yT[slice, :] = M[:, slice]^T @ h^T ----
        # transposed orientation: stationary = M column block, moving = hT
        # 512-row chunks — half the matmul count of the y-major form. The
        # host transposes yT back when scattering output columns.
        for lc in range(8):
            for cb in range(2):
                psy = pacc.tile([128, 512], F32, name="psy", tag="acc", bufs=4)
                for ib in range(8):
                    nc.tensor.matmul(
                        psy[:],
                        mq[:, ib, cb * 128:(cb + 1) * 128],
                        hT[:, ib, lc * 512:(lc + 1) * 512],
                        start=(ib == 0), stop=(ib == 7),
                    )
                ysb = stream.tile([128, 512], BF16, name="ysb", tag="yo", bufs=4)
                last = lc == 7 and cb == 1
                if last:
                    # split the final drain across both engines + two DMAs
                    # on separate SEQ rings so the issue chains overlap
                    nc.scalar.copy(ysb[:, 0:256], psy[:, 0:256])
                    nc.vector.tensor_copy(ysb[:, 256:512], psy[:, 256:512])
                    nc.scalar.dma_start(
                        y_ap[cb * 128:(cb + 1) * 128,
                             lc * 512:lc * 512 + 256], ysb[:, 0:256])
                    nc.sync.dma_start(
                        y_ap[cb * 128:(cb + 1) * 128,
                             lc * 512 + 256:(lc + 1) * 512], ysb[:, 256:512])
                else:
                    if (lc * 2 + cb) % 2 == 0:
                        nc.scalar.copy(ysb[:], psy[:])
                    else:
                        nc.vector.tensor_copy(ysb[:], psy[:])
                    nc.scalar.dma_start(
                        y_ap[cb * 128:(cb + 1) * 128, lc * 512:(lc + 1) * 512],
                        ysb[:],
                    )


_NC_CACHE = {}


def _build():
    if "nc" in _NC_CACHE:
        return _NC_CACHE["nc"]
    nc = bacc.Bacc("TRN2", target_bir_lowering=False, debug=False, num_devices=8)
    hcs_d = nc.dram_tensor("hcs", [L, 2 * H], BF16, kind="ExternalInput")
    wq_d = nc.dram_tensor("wq", [H, H], BF16, kind="ExternalInput")
    y_d = nc.dram_tensor("y", [NCS, L], BF16, kind="ExternalOutput")
    with tile.TileContext(nc) as tc:
        _emit_once(nc, tc, hcs_d, wq_d, y_d)
    nc.compile()
    _NC_CACHE["nc"] = nc
    return nc


def _prep_inputs(hidden_states, W_q, cos, sin):
    """Shard/permute on host: per core j, roll the RoPE pair axis by 128*j
    so the core's G/M/y column slice sits at permuted cols {0:128,512:640};
    W_q gets the same permutation on both axes. Pack [h|cos|sin] into one
    row-major bf16 tensor so the stream is one DMA per l-tile."""
    hs = np.asarray(hidden_states, dtype=np.float32)
    wq = np.asarray(W_q, dtype=np.float32)
    c1 = np.asarray(cos, dtype=np.float32)[:, :HH]
    s1 = np.asarray(sin, dtype=np.float32)[:, :HH]

    in_maps = []
    for core in range(8):
        b, j = core // 4, core % 4
        pr = (np.arange(HH) + PAIRS * j) % HH          # pair permutation
        idx = np.concatenate([pr, HH + pr])            # full column perm
        hp = hs[b][:, idx]
        hcs = np.concatenate([hp, c1[:, pr], s1[:, pr]], axis=1)
        wp = wq[np.ix_(idx, idx)]
        in_maps.append({
            "hcs": np.ascontiguousarray(hcs.astype(BF16_NP)),
            "wq": np.ascontiguousarray(wp.astype(BF16_NP)),
        })
    return in_maps


def kernel(hidden_states, W_q, cos, sin):
    in_maps = _prep_inputs(hidden_states, W_q, cos, sin)
    nc = _build()
    res = run_bass_kernel_spmd(nc, in_maps, core_ids=list(range(8)))

    out = np.empty((B, L, H), dtype=np.float32)
    for core, r in enumerate(res.results):
        b, j = core // 4, core % 4
        yt = np.asarray(r["y"]).astype(np.float32)     # [256, L] packed lo|hi
        out[b][:, PAIRS * j:PAIRS * (j + 1)] = yt[0:PAIRS].T
        out[b][:, HH + PAIRS * j:HH + PAIRS * (j + 1)] = yt[PAIRS:NCS].T
    return out
